# revision 24
# baseline (speedup 1.0000x reference)
"""BiaffineAttn Trainium2 kernel.

Math (per batch b):
    t    = x2 @ U + bias[None, :]      [S, D]   (bias folded: x2 U x1^T + 1 (x1 bias)^T
                                                 == [x2|1] [[U];[bias^T]] x1^T)
    attn = t @ x1^T
    p    = softmax(attn, axis=-1)
    out  = relu((p @ x1) @ fc_w^T + fc_b)       [S, F]

Sharding: data-parallel over batch B=8, one batch per NeuronCore.

Per-core pipeline, transposed orientation (softmax key dim t' on partitions),
software-pipelined across S-superblocks so the PE never idles during softmax.
The interleave window overlays three instruction streams per t'-tile:
  PE:     MM2(sb+1) tile (8 mm, 4-buf PSUM ring) + one MM1(sb+2) dc-step
          (4 mm into 4 held PSUM banks)  -> 12 mm/tile keeps PE the pacer
  Scalar: exp(sb, tile) + MM2 psum->SBUF score copy
  DVE:    running max + softmax-denominator accumulation
  GpSimd: score - maxb subtract (SBUF-only; Pool cannot touch PSUM)
Block 2 has no MM1 to interleave, so half of MM4(2) (et 0-3, ti-outer) fills
the window; block 3 interleaves the full ti-outer MM4(3) in 8 held banks.

rowsum via single ones-column matmul; reciprocal_approx_fast; relu+bias via
Scalar activation; [F,SB] stores per superblock.

All tensors are host-relaid so every resident loads in ONE big DMA (32KB
rows) and each x1t t'-group is a single [128, 4096] transfer (2-deep ring).

Host side: builds the relaid views and transposes the [F,S] per-core output
back to [S,F] when gathering (fp32 DMA transpose does not exist on TRN2).
"""

import os
import sys
from contextlib import ExitStack

import numpy as np

for _p in ("/opt/trn_rl_repo", os.path.expanduser("~/.axon_site/_ro/trn_rl_repo")):
    if os.path.isdir(_p) and _p not in sys.path:
        sys.path.insert(0, _p)

import concourse.bass as bass
import concourse.mybir as mybir
import concourse.tile as tile
from concourse import bacc

B = 8
S = 2048          # sequence length (both s and t')
D = 1024          # d_model
F = 512           # fc output dim
P = 128
SB = 512          # s superblock (moving free dim of every matmul)
NSB = S // SB     # 4
DC = D // P       # 8 contraction chunks of d / e
TC = S // P       # 16 t' tiles
NTG = NSB         # 4 t' groups of 4 tiles
FT = F // P       # 4
FP32 = mybir.dt.float32
FP32R = mybir.dt.float32r
BF16 = mybir.dt.bfloat16
AF = mybir.ActivationFunctionType
ALU = mybir.AluOpType
AX = mybir.AxisListType

OT_DT = BF16      # dtype of MM4 output tiles (MM5 moving operand)


def build_nc():
    nc = bacc.Bacc(
        "TRN2",
        target_bir_lowering=False,
        debug=False,
        enable_asserts=False,
    )

    # host-relaid tensors: row p holds the p-th partition's data for every tile
    x1_d = nc.dram_tensor("x1g", [P, TC * D], BF16, kind="ExternalInput")
    x1t_d = nc.dram_tensor("x1tg", [P, NTG * DC * SB], FP32R, kind="ExternalInput")
    x2t_d = nc.dram_tensor("x2t", [D, S], FP32R, kind="ExternalInput")
    u_d = nc.dram_tensor("ug", [P, DC * D], FP32R, kind="ExternalInput")
    fcwt_d = nc.dram_tensor("fcwg", [P, DC * F], BF16, kind="ExternalInput")
    bias_d = nc.dram_tensor("biasg", [P, DC], FP32, kind="ExternalInput")
    fcb_d = nc.dram_tensor("fcbg", [P, FT], FP32, kind="ExternalInput")
    outt_d = nc.dram_tensor("outt", [F, S], FP32, kind="ExternalOutput")

    with tile.TileContext(nc) as tc, ExitStack() as ctx:
        # ---------- pools ----------
        p_u = ctx.enter_context(tc.tile_pool(name="ures", bufs=1))
        p_x1 = ctx.enter_context(tc.tile_pool(name="x1res", bufs=1))
        p_bc = ctx.enter_context(tc.tile_pool(name="biascols", bufs=1))
        p_fcb = ctx.enter_context(tc.tile_pool(name="fcbcols", bufs=1))
        p_fcw = ctx.enter_context(tc.tile_pool(name="fcwres", bufs=1))
        p_ones = ctx.enter_context(tc.tile_pool(name="ones", bufs=1))
        p_psum = ctx.enter_context(tc.tile_pool(name="psum", bufs=4, space="PSUM"))
        p_psum1 = ctx.enter_context(tc.tile_pool(name="psum1", bufs=4, space="PSUM"))
        p_x2t = ctx.enter_context(tc.tile_pool(name="x2ts", bufs=5))
        p_x1tc = ctx.enter_context(tc.tile_pool(name="x1tgs", bufs=2))
        p_tt = ctx.enter_context(tc.tile_pool(name="tts", bufs=9))
        p_sc = ctx.enter_context(tc.tile_pool(name="scores", bufs=TC))
        p_pb = ctx.enter_context(tc.tile_pool(name="pbf", bufs=TC))
        p_ot = ctx.enter_context(tc.tile_pool(name="ots", bufs=DC))
        p_aux = ctx.enter_context(tc.tile_pool(name="aux", bufs=1))
        p_row = ctx.enter_context(tc.tile_pool(name="rows", bufs=1))
        p_tmp = ctx.enter_context(tc.tile_pool(name="tmps", bufs=1))
        p_oo = ctx.enter_context(tc.tile_pool(name="oos", bufs=1))

        # ---------- prologue DMAs: MM1(0) critical path first ----------
        u_big = p_u.tile([P, DC * D], FP32R, name="ug", tag="ur")
        for q in range(4):
            nc.sync.dma_start(
                u_big[:, q * 2 * D : (q + 1) * 2 * D],
                u_d[:, q * 2 * D : (q + 1) * 2 * D],
            )

        def u_sl(dc, et):
            return u_big[:, dc * D + et * P : dc * D + (et + 1) * P]

        x2t_tiles = {0: []}
        for dc in range(DC):
            x2_t = p_x2t.tile([P, SB], FP32R, name=f"x2t0_{dc}", tag="x2t")
            nc.sync.dma_start(x2_t[:], x2t_d[dc * P : (dc + 1) * P, 0:SB])
            x2t_tiles[0].append(x2_t)

        # x1t t'-group streaming (MM2 stationaries): one DMA per group, ring 2
        x1tg = {}

        def issue_x1tg(key, tg):
            # gpsimd queue: ring-WAR blocked group loads must not head-of-line
            # block the sync queue
            t = p_x1tc.tile([P, DC * SB], FP32R, name=f"x1tg{key}", tag="x1tg")
            nc.gpsimd.dma_start(t[:], x1t_d[:, tg * DC * SB : (tg + 1) * DC * SB])
            x1tg[key] = t

        issue_x1tg(0, 0)
        issue_x1tg(1, 1)
        bias_cols = p_bc.tile([P, DC], FP32, name="bc", tag="bc")
        nc.sync.dma_start(bias_cols[:], bias_d[:, :])

        identity32 = p_ones.tile([P, P], FP32, name="ident32", tag="ident32")
        nc.gpsimd.memset(identity32[:], 0.0)
        nc.gpsimd.affine_select(
            out=identity32[:], in_=identity32[:], compare_op=ALU.not_equal,
            fill=1.0, base=0, pattern=[[-1, P]], channel_multiplier=1,
        )
        identity = p_ones.tile([P, P], FP32R, name="ident", tag="ident")
        nc.scalar.activation(identity[:], identity32[:], AF.Identity, bias=0.0, scale=1.0)
        ones_row = p_ones.tile([1, P], FP32R, name="ones_row", tag="ones_row")
        nc.scalar.activation(ones_row[:], identity[0:1, :], AF.Identity, bias=1.0, scale=0.0)
        ones_colb = p_ones.tile([P, 1], BF16, name="ones_colb", tag="ones_colb")
        nc.scalar.activation(ones_colb[:], identity[:, 0:1], AF.Identity, bias=1.0, scale=0.0)
        ones_row32 = p_ones.tile([1, P], FP32, name="ones_row32", tag="ones_row32")
        nc.scalar.activation(ones_row32[:], identity[0:1, :], AF.Identity, bias=1.0, scale=0.0)

        def issue_x2t(sb):
            tiles = []
            for dc in range(DC):
                x2_t = p_x2t.tile([P, SB], FP32R, name=f"x2t{sb}_{dc}", tag="x2t")
                nc.sync.dma_start(
                    x2_t[:], x2t_d[dc * P : (dc + 1) * P, sb * SB : (sb + 1) * SB]
                )
                tiles.append(x2_t)
            return tiles

        x2t_tiles[1] = issue_x2t(1)

        # residents (needed from MM4(0)/MM5(0) on) issued after all critical
        # prologue streams
        x1_big = p_x1.tile([P, TC * D], BF16, name="x1g", tag="x1r")
        nc.sync.dma_start(x1_big[:], x1_d[:, :])

        def x1_sl(ti, et):
            return x1_big[:, ti * D + et * P : ti * D + (et + 1) * P]

        fcw_big = p_fcw.tile([P, DC * F], BF16, name="fcwg", tag="fcw")
        nc.sync.dma_start(fcw_big[:], fcwt_d[:, :])

        def fcw_sl(ec, ft):
            return fcw_big[:, ec * F + ft * P : ec * F + (ft + 1) * P]

        fcb_cols = p_fcb.tile([P, FT], FP32, name="fcb", tag="fcb")
        nc.sync.dma_start(fcb_cols[:], fcb_d[:, :])

        # ---------- MM1: ttT = (x2 @ U)^T + bias, 8 held banks (both pools) ----------
        tt = {}

        def emit_mm1(sb):
            tt[sb] = []
            ps = [
                p_psum1.tile([P, SB], FP32, name=f"ps1_{sb}_{et}", tag="ps1")
                for et in range(4)
            ] + [
                p_psum.tile([P, SB], FP32, name=f"ps1b_{sb}_{et}", tag="ps")
                for et in range(4, DC)
            ]
            for dc in range(DC):
                for et in range(DC):
                    nc.tensor.matmul(
                        ps[et][:], u_sl(dc, et), x2t_tiles[sb][dc][:],
                        start=(dc == 0), stop=(dc == DC - 1),
                    )
            for et in range(DC):
                t_t = p_tt.tile([P, SB], FP32R, name=f"tt{sb}_{et}", tag="tt")
                nc.scalar.activation(
                    t_t[:], ps[et][:], AF.Identity,
                    bias=bias_cols[:, et : et + 1], scale=1.0,
                )
                tt[sb].append(t_t)

        emit_mm1(0)

        # ---------- MM2 per-tile emitter: scoresT tile + running max ----------
        sc_tiles = {}
        maxacc = {}
        sumacc = {}
        pb_tiles = {}

        def emit_mm2_tile(sb, ti):
            tg, sub = divmod(ti, SB // P)
            if ti == 0:
                sc_tiles[sb] = []
                maxacc[sb] = p_aux.tile(
                    [P, SB], FP32R, name=f"maxacc{sb}", tag="maxacc"
                )
            ps_s = p_psum.tile([P, SB], FP32, name=f"pss{sb}_{ti}", tag="ps")
            grp = x1tg[sb * NTG + tg]
            for ec in range(DC):
                nc.tensor.matmul(
                    ps_s[:],
                    grp[:, ec * SB + sub * P : ec * SB + (sub + 1) * P],
                    tt[sb][ec][:],
                    start=(ec == 0), stop=(ec == DC - 1),
                )
            s_t = p_sc.tile([P, SB], FP32, name=f"sc{sb}_{ti}", tag="sc")
            nc.scalar.copy(s_t[:], ps_s[:])
            if ti == 0:
                nc.vector.tensor_copy(maxacc[sb][:], s_t[:])
            else:
                nc.vector.tensor_max(maxacc[sb][:], maxacc[sb][:], s_t[:])
            sc_tiles[sb].append(s_t)

        # prologue MM2(0) with group prefetch, then MM1(1)
        for ti in range(TC):
            tg, sub = divmod(ti, SB // P)
            if sub == 0 and tg + 2 < NTG:
                issue_x1tg(tg + 2, tg + 2)
            emit_mm2_tile(0, ti)
        emit_mm1(1)
        maxbs = {}

        # ---------- softmax helpers ----------
        def emit_max_reduce(sb):
            """per-s max over partitions -> maxb [P,SB] broadcast tile."""
            mrow = p_row.tile([1, SB], FP32R, name=f"mrow{sb}", tag="mrow")
            trs, mcols = [], []
            for blk in range(SB // P):
                ps_tr = p_psum.tile([P, P], FP32R, name=f"ptr{sb}_{blk}", tag="ps")
                nc.tensor.transpose(
                    ps_tr[:], maxacc[sb][:, blk * P : (blk + 1) * P], identity[:]
                )
                trs.append(ps_tr)
            for blk in range(SB // P):
                mcol = p_row.tile([P, 1], FP32R, name=f"mcol{sb}_{blk}", tag=f"mcol{blk % 2}")
                nc.vector.reduce_max(mcol[:], trs[blk][:], axis=AX.X)
                mcols.append(mcol)
            ps_rrs = []
            for blk in range(SB // P):
                ps_rr = p_psum.tile([1, P], FP32R, name=f"prr{sb}_{blk}", tag="ps")
                nc.tensor.transpose(ps_rr[:], mcols[blk][:], identity[:])
                ps_rrs.append(ps_rr)
            for blk in range(SB // P):
                nc.vector.tensor_copy(mrow[:, blk * P : (blk + 1) * P], ps_rrs[blk][:])
            ps_mb = p_psum.tile([P, SB], FP32, name=f"pmb{sb}", tag="ps")
            nc.tensor.matmul(ps_mb[:], ones_row[:], mrow[:], start=True, stop=True)
            maxb = p_aux.tile([P, SB], FP32, name=f"maxb{sb}", tag="maxb")
            nc.vector.tensor_copy(maxb[:], ps_mb[:])
            return maxb

        def emit_exp_tile(sb, ti, maxb):
            if ti == 0:
                pb_tiles[sb] = []
            s_t = sc_tiles[sb][ti]
            nc.vector.tensor_sub(s_t[:], s_t[:], maxb[:])
            p_t = p_pb.tile([P, SB], BF16, name=f"pb{sb}_{ti}", tag="pb")
            nc.scalar.activation(p_t[:], s_t[:], AF.Exp, bias=0.0, scale=1.0)
            pb_tiles[sb].append(p_t)

        def emit_sum_mm(sb, ti, ps_sum):
            """accumulate softmax denominator on the PE: [1,SB] += 1^T @ pb."""
            nc.tensor.matmul(
                ps_sum[:], ones_colb[:], pb_tiles[sb][ti][:],
                start=(ti == 0), stop=(ti == TC - 1),
            )

        def emit_sum_recip(sb, ps_sum):
            rrow = p_row.tile([1, SB], FP32, name=f"rrow{sb}", tag="rrow")
            with nc.allow_low_precision(reason="softmax denom reciprocal; fp22 ok"):
                nc.vector.reciprocal_approx_fast(rrow[:], ps_sum[:])
            return rrow

        def emit_recip_bcast(sb, rrow):
            ps_rb = p_psum.tile([P, SB], FP32, name=f"prb{sb}", tag="ps")
            nc.tensor.matmul(ps_rb[:], ones_row32[:], rrow[:], start=True, stop=True)
            recipb = p_aux.tile([P, SB], FP32, name=f"recipb{sb}", tag="recipb")
            nc.vector.tensor_copy(recipb[:], ps_rb[:])
            return recipb

        def mm4_copy_out(sb, ps_list, ots, et0):
            for i, ps_o in enumerate(ps_list):
                o_t = p_ot.tile([P, SB], OT_DT, name=f"ot{sb}_{et0 + i}", tag="ot")
                nc.vector.tensor_copy(o_t[:], ps_o[:])
                ots.append(o_t)

        def emit_mm5(sb, ots, recipb):
            s0 = sb * SB
            for ft in range(FT):
                ps_f = p_psum.tile([P, SB], FP32, name=f"psf{sb}_{ft}", tag="ps")
                for ec in range(DC):
                    nc.tensor.matmul(
                        ps_f[:], fcw_sl(ec, ft), ots[ec][:],
                        start=(ec == 0), stop=(ec == DC - 1),
                    )
                tmp = p_tmp.tile([P, SB], FP32, name=f"tmp{sb}_{ft}", tag="tmp")
                nc.vector.tensor_mul(tmp[:], ps_f[:], recipb[:])
                o_out = p_oo.tile([P, SB], FP32, name=f"oo{sb}_{ft}", tag="oo")
                nc.scalar.activation(
                    o_out[:], tmp[:], AF.Relu,
                    bias=fcb_cols[:, ft : ft + 1], scale=1.0,
                )
                nc.sync.dma_start(outt_d[ft * P : (ft + 1) * P, s0 : s0 + SB], o_out[:])

        # ---------- steady-state blocks ----------
        maxbs[0] = emit_max_reduce(0)
        for sb in range(NSB):
            if sb + 2 < NSB:
                x2t_tiles[sb + 2] = issue_x2t(sb + 2)
            if sb + 1 < NSB:
                issue_x1tg((sb + 1) * NTG, 0)

            maxb = maxbs[sb]

            if sb + 1 < NSB:
                # PE chews MM2(sb+1) + denom matmuls while Scalar/DVE run exp(sb)
                ps_sum = p_psum1.tile([1, SB], FP32, name=f"psum{sb}", tag="ps1")
                for ti in range(TC):
                    emit_exp_tile(sb, ti, maxb)
                    tg, sub = divmod(ti, SB // P)
                    if sub == 0 and tg + 1 < NTG:
                        issue_x1tg((sb + 1) * NTG + tg + 1, tg + 1)
                    emit_mm2_tile(sb + 1, ti)
                    emit_sum_mm(sb, ti, ps_sum)
                rrow = emit_sum_recip(sb, ps_sum)
                if sb + 2 < NSB:
                    emit_mm1(sb + 2)
                # maxT(sb+1) here: the PE<->DVE ping-pong hides under MM4/MM5
                maxbs[sb + 1] = emit_max_reduce(sb + 1)
                ots = []
                for et in range(DC):
                    ps_o = p_psum.tile([P, SB], FP32, name=f"pso{sb}_{et}", tag="ps")
                    for ti in range(TC):
                        nc.tensor.matmul(
                            ps_o[:], x1_sl(ti, et), pb_tiles[sb][ti][:],
                            start=(ti == 0), stop=(ti == TC - 1),
                        )
                    mm4_copy_out(sb, [ps_o], ots, et)
                recipb = emit_recip_bcast(sb, rrow)
                emit_mm5(sb, ots, recipb)
            else:
                # last block: ti-outer MM4 on 7 held banks + denom bank; et7 after
                mm4_ps = [
                    p_psum1.tile([P, SB], FP32, name=f"pso{sb}_{et}", tag="ps1")
                    for et in range(4)
                ] + [
                    p_psum.tile([P, SB], FP32, name=f"pso{sb}_{et}", tag="ps")
                    for et in range(4, DC - 1)
                ]
                ps_sum = p_psum.tile([1, SB], FP32, name=f"psum{sb}", tag="ps")
                for ti in range(TC):
                    emit_exp_tile(sb, ti, maxb)
                    for et in range(DC - 1):
                        nc.tensor.matmul(
                            mm4_ps[et][:], x1_sl(ti, et), pb_tiles[sb][ti][:],
                            start=(ti == 0), stop=(ti == TC - 1),
                        )
                    emit_sum_mm(sb, ti, ps_sum)
                ots = []
                mm4_copy_out(sb, mm4_ps, ots, 0)
                rrow = emit_sum_recip(sb, ps_sum)
                ps_o7 = p_psum.tile([P, SB], FP32, name=f"pso{sb}_7", tag="ps")
                for ti in range(TC):
                    nc.tensor.matmul(
                        ps_o7[:], x1_sl(ti, DC - 1), pb_tiles[sb][ti][:],
                        start=(ti == 0), stop=(ti == TC - 1),
                    )
                mm4_copy_out(sb, [ps_o7], ots, DC - 1)
                recipb = emit_recip_bcast(sb, rrow)
                emit_mm5(sb, ots, recipb)

    nc.compile()
    return nc


_NC_CACHE = None


def _get_nc():
    global _NC_CACHE
    if _NC_CACHE is None:
        _NC_CACHE = build_nc()
    return _NC_CACHE


def make_in_maps(x1, x2, U, bias, fc_w, fc_b):
    import ml_dtypes

    x1 = np.ascontiguousarray(np.asarray(x1, dtype=np.float32))
    x2 = np.ascontiguousarray(np.asarray(x2, dtype=np.float32))
    U = np.ascontiguousarray(np.asarray(U, dtype=np.float32))
    bias = np.asarray(bias, dtype=np.float32)
    fc_w = np.asarray(fc_w, dtype=np.float32)
    fc_b = np.asarray(fc_b, dtype=np.float32)
    # relaid residents (same for every core)
    ug = np.ascontiguousarray(
        U.reshape(DC, P, D).transpose(1, 0, 2).reshape(P, DC * D)
    )
    fcwg = np.ascontiguousarray(
        fc_w.T.reshape(DC, P, F).transpose(1, 0, 2).reshape(P, DC * F)
    ).astype(ml_dtypes.bfloat16)
    biasg = np.ascontiguousarray(bias.reshape(DC, P).T)
    fcbg = np.ascontiguousarray(fc_b.reshape(FT, P).T)
    in_maps = []
    for b in range(B):
        x1t = x1[b].T  # [D, S]
        x1tg = np.ascontiguousarray(
            x1t.reshape(DC, P, NTG, SB).transpose(1, 2, 0, 3).reshape(P, NTG * DC * SB)
        )
        x1g = np.ascontiguousarray(
            x1[b].reshape(TC, P, D).transpose(1, 0, 2).reshape(P, TC * D)
        ).astype(ml_dtypes.bfloat16)
        in_maps.append(
            {
                "x1g": x1g,
                "x1tg": x1tg,
                "x2t": np.ascontiguousarray(x2[b].T),
                "ug": ug,
                "fcwg": fcwg,
                "biasg": biasg,
                "fcbg": fcbg,
            }
        )
    return in_maps


def kernel(x1, x2, U, bias, fc_w, fc_b):
    from concourse.bass_utils import run_bass_kernel_spmd

    nc = _get_nc()
    in_maps = make_in_maps(x1, x2, U, bias, fc_w, fc_b)
    res = run_bass_kernel_spmd(nc, in_maps, core_ids=list(range(B)))
    out = np.stack([np.ascontiguousarray(r["outt"].T) for r in res.results])
    return out.astype(np.float32)


# revision 27
# speedup vs baseline: 1.0226x; 1.0226x over previous
"""BiaffineAttn Trainium2 kernel.

Math (per batch b):
    t    = x2 @ U + bias[None, :]      [S, D]   (bias folded: x2 U x1^T + 1 (x1 bias)^T
                                                 == [x2|1] [[U];[bias^T]] x1^T)
    attn = t @ x1^T
    p    = softmax(attn, axis=-1)
    out  = relu((p @ x1) @ fc_w^T + fc_b)       [S, F]

Sharding: data-parallel over batch B=8, one batch per NeuronCore.

Per-core pipeline, transposed orientation (softmax key dim t' on partitions),
software-pipelined across S-superblocks so the PE never idles during softmax.
The interleave window overlays three instruction streams per t'-tile:
  PE:     MM2(sb+1) tile (8 mm, 4-buf PSUM ring) + one MM1(sb+2) dc-step
          (4 mm into 4 held PSUM banks)  -> 12 mm/tile keeps PE the pacer
  Scalar: exp(sb, tile) + MM2 psum->SBUF score copy
  DVE:    running max + softmax-denominator accumulation
  GpSimd: score - maxb subtract (SBUF-only; Pool cannot touch PSUM)
Block 2 has no MM1 to interleave, so half of MM4(2) (et 0-3, ti-outer) fills
the window; block 3 interleaves the full ti-outer MM4(3) in 8 held banks.

rowsum via single ones-column matmul; reciprocal_approx_fast; relu+bias via
Scalar activation; [F,SB] stores per superblock.

All tensors are host-relaid so every resident loads in ONE big DMA (32KB
rows) and each x1t t'-group is a single [128, 4096] transfer (2-deep ring).

Host side: builds the relaid views and transposes the [F,S] per-core output
back to [S,F] when gathering (fp32 DMA transpose does not exist on TRN2).
"""

import os
import sys
from contextlib import ExitStack

import numpy as np

for _p in ("/opt/trn_rl_repo", os.path.expanduser("~/.axon_site/_ro/trn_rl_repo")):
    if os.path.isdir(_p) and _p not in sys.path:
        sys.path.insert(0, _p)

import concourse.bass as bass
import concourse.mybir as mybir
import concourse.tile as tile
from concourse import bacc

B = 8
S = 2048          # sequence length (both s and t')
D = 1024          # d_model
F = 512           # fc output dim
P = 128
SB = 512          # s superblock (moving free dim of every matmul)
NSB = S // SB     # 4
DC = D // P       # 8 contraction chunks of d / e
TC = S // P       # 16 t' tiles
NTG = NSB         # 4 t' groups of 4 tiles
FT = F // P       # 4
FP32 = mybir.dt.float32
FP32R = mybir.dt.float32r
BF16 = mybir.dt.bfloat16
AF = mybir.ActivationFunctionType
ALU = mybir.AluOpType
AX = mybir.AxisListType

OT_DT = BF16      # dtype of MM4 output tiles (MM5 moving operand)


def build_nc():
    nc = bacc.Bacc(
        "TRN2",
        target_bir_lowering=False,
        debug=False,
        enable_asserts=False,
    )

    # host-relaid tensors: row p holds the p-th partition's data for every tile
    x1_d = nc.dram_tensor("x1g", [P, TC * D], BF16, kind="ExternalInput")
    x1t_d = nc.dram_tensor("x1tg", [P, NTG * DC * SB], FP32R, kind="ExternalInput")
    x2t_d = nc.dram_tensor("x2t", [D, S], FP32R, kind="ExternalInput")
    u_d = nc.dram_tensor("ug", [P, DC * D], FP32R, kind="ExternalInput")
    fcwt_d = nc.dram_tensor("fcwg", [P, DC * F], BF16, kind="ExternalInput")
    bias_d = nc.dram_tensor("biasg", [P, DC], FP32, kind="ExternalInput")
    fcb_d = nc.dram_tensor("fcbg", [P, FT], FP32, kind="ExternalInput")
    outt_d = nc.dram_tensor("outt", [F, S], FP32, kind="ExternalOutput")

    with tile.TileContext(nc) as tc, ExitStack() as ctx:
        # ---------- pools ----------
        p_u = ctx.enter_context(tc.tile_pool(name="ures", bufs=1))
        p_x1 = ctx.enter_context(tc.tile_pool(name="x1res", bufs=1))
        p_bc = ctx.enter_context(tc.tile_pool(name="biascols", bufs=1))
        p_fcb = ctx.enter_context(tc.tile_pool(name="fcbcols", bufs=1))
        p_fcw = ctx.enter_context(tc.tile_pool(name="fcwres", bufs=1))
        p_ones = ctx.enter_context(tc.tile_pool(name="ones", bufs=1))
        p_psum = ctx.enter_context(tc.tile_pool(name="psum", bufs=4, space="PSUM"))
        p_psum1 = ctx.enter_context(tc.tile_pool(name="psum1", bufs=4, space="PSUM"))
        p_x2t = ctx.enter_context(tc.tile_pool(name="x2ts", bufs=5))
        p_x1tc = ctx.enter_context(tc.tile_pool(name="x1tgs", bufs=2))
        p_tt = ctx.enter_context(tc.tile_pool(name="tts", bufs=9))
        p_sc = ctx.enter_context(tc.tile_pool(name="scores", bufs=TC))
        p_pb = ctx.enter_context(tc.tile_pool(name="pbf", bufs=TC))
        p_ot = ctx.enter_context(tc.tile_pool(name="ots", bufs=DC))
        p_aux = ctx.enter_context(tc.tile_pool(name="aux", bufs=1))
        p_row = ctx.enter_context(tc.tile_pool(name="rows", bufs=1))
        p_tmp = ctx.enter_context(tc.tile_pool(name="tmps", bufs=1))
        p_oo = ctx.enter_context(tc.tile_pool(name="oos", bufs=1))

        # ---------- prologue DMAs: MM1(0) critical path first ----------
        u_big = p_u.tile([P, DC * D], FP32R, name="ug", tag="ur")
        for q in range(4):
            nc.sync.dma_start(
                u_big[:, q * 2 * D : (q + 1) * 2 * D],
                u_d[:, q * 2 * D : (q + 1) * 2 * D],
            )

        def u_sl(dc, et):
            return u_big[:, dc * D + et * P : dc * D + (et + 1) * P]

        # prologue x2t on the Scalar HWDGE queue: overlaps with u on sync
        x2t_tiles = {0: []}
        for dc in range(DC):
            x2_t = p_x2t.tile([P, SB], FP32R, name=f"x2t0_{dc}", tag="x2t")
            nc.scalar.dma_start(x2_t[:], x2t_d[dc * P : (dc + 1) * P, 0:SB])
            x2t_tiles[0].append(x2_t)

        # x1t t'-group streaming (MM2 stationaries): one DMA per group, ring 2
        x1tg = {}

        def issue_x1tg(key, tg):
            # gpsimd queue: ring-WAR blocked group loads must not head-of-line
            # block the sync queue
            t = p_x1tc.tile([P, DC * SB], FP32R, name=f"x1tg{key}", tag="x1tg")
            nc.gpsimd.dma_start(t[:], x1t_d[:, tg * DC * SB : (tg + 1) * DC * SB])
            x1tg[key] = t

        issue_x1tg(0, 0)
        issue_x1tg(1, 1)
        bias_cols = p_bc.tile([P, DC], FP32, name="bc", tag="bc")
        nc.scalar.dma_start(bias_cols[:], bias_d[:, :])

        identity32 = p_ones.tile([P, P], FP32, name="ident32", tag="ident32")
        nc.gpsimd.memset(identity32[:], 0.0)
        nc.gpsimd.affine_select(
            out=identity32[:], in_=identity32[:], compare_op=ALU.not_equal,
            fill=1.0, base=0, pattern=[[-1, P]], channel_multiplier=1,
        )
        identity = p_ones.tile([P, P], FP32R, name="ident", tag="ident")
        nc.scalar.activation(identity[:], identity32[:], AF.Identity, bias=0.0, scale=1.0)
        ones_row = p_ones.tile([1, P], FP32R, name="ones_row", tag="ones_row")
        nc.scalar.activation(ones_row[:], identity[0:1, :], AF.Identity, bias=1.0, scale=0.0)
        ones_colb = p_ones.tile([P, 1], BF16, name="ones_colb", tag="ones_colb")
        nc.scalar.activation(ones_colb[:], identity[:, 0:1], AF.Identity, bias=1.0, scale=0.0)
        ones_row32 = p_ones.tile([1, P], FP32, name="ones_row32", tag="ones_row32")
        nc.scalar.activation(ones_row32[:], identity[0:1, :], AF.Identity, bias=1.0, scale=0.0)

        def issue_x2t(sb, eng=None):
            eng = eng or nc.sync
            tiles = []
            for dc in range(DC):
                x2_t = p_x2t.tile([P, SB], FP32R, name=f"x2t{sb}_{dc}", tag="x2t")
                eng.dma_start(
                    x2_t[:], x2t_d[dc * P : (dc + 1) * P, sb * SB : (sb + 1) * SB]
                )
                tiles.append(x2_t)
            return tiles

        x2t_tiles[1] = issue_x2t(1, nc.scalar)

        # residents (needed from MM4(0)/MM5(0) on) issued after all critical
        # prologue streams
        x1_big = p_x1.tile([P, TC * D], BF16, name="x1g", tag="x1r")
        nc.sync.dma_start(x1_big[:, : TC * D // 2], x1_d[:, : TC * D // 2])
        nc.sync.dma_start(x1_big[:, TC * D // 2 :], x1_d[:, TC * D // 2 :])

        def x1_sl(ti, et):
            return x1_big[:, ti * D + et * P : ti * D + (et + 1) * P]

        fcw_big = p_fcw.tile([P, DC * F], BF16, name="fcwg", tag="fcw")
        nc.sync.dma_start(fcw_big[:], fcwt_d[:, :])

        def fcw_sl(ec, ft):
            return fcw_big[:, ec * F + ft * P : ec * F + (ft + 1) * P]

        fcb_cols = p_fcb.tile([P, FT], FP32, name="fcb", tag="fcb")
        nc.sync.dma_start(fcb_cols[:], fcb_d[:, :])

        # ---------- MM1: ttT = (x2 @ U)^T + bias, 8 held banks (both pools) ----------
        tt = {}

        def emit_mm1(sb):
            tt[sb] = []
            ps = [
                p_psum1.tile([P, SB], FP32, name=f"ps1_{sb}_{et}", tag="ps1")
                for et in range(4)
            ] + [
                p_psum.tile([P, SB], FP32, name=f"ps1b_{sb}_{et}", tag="ps")
                for et in range(4, DC)
            ]
            for dc in range(DC):
                for et in range(DC):
                    nc.tensor.matmul(
                        ps[et][:], u_sl(dc, et), x2t_tiles[sb][dc][:],
                        start=(dc == 0), stop=(dc == DC - 1),
                    )
            for et in range(DC):
                t_t = p_tt.tile([P, SB], FP32R, name=f"tt{sb}_{et}", tag="tt")
                nc.scalar.activation(
                    t_t[:], ps[et][:], AF.Identity,
                    bias=bias_cols[:, et : et + 1], scale=1.0,
                )
                tt[sb].append(t_t)

        emit_mm1(0)

        # ---------- MM2 per-tile emitter: scoresT tile + running max ----------
        sc_tiles = {}
        maxacc = {}
        sumacc = {}
        pb_tiles = {}

        def emit_mm2_tile(sb, ti):
            tg, sub = divmod(ti, SB // P)
            if ti == 0:
                sc_tiles[sb] = []
                maxacc[sb] = p_aux.tile(
                    [P, SB], FP32R, name=f"maxacc{sb}", tag="maxacc"
                )
            ps_s = p_psum.tile([P, SB], FP32, name=f"pss{sb}_{ti}", tag="ps")
            grp = x1tg[sb * NTG + tg]
            for ec in range(DC):
                nc.tensor.matmul(
                    ps_s[:],
                    grp[:, ec * SB + sub * P : ec * SB + (sub + 1) * P],
                    tt[sb][ec][:],
                    start=(ec == 0), stop=(ec == DC - 1),
                )
            s_t = p_sc.tile([P, SB], FP32, name=f"sc{sb}_{ti}", tag="sc")
            nc.scalar.copy(s_t[:], ps_s[:])
            if ti == 0:
                nc.vector.tensor_copy(maxacc[sb][:], s_t[:])
            else:
                nc.vector.tensor_max(maxacc[sb][:], maxacc[sb][:], s_t[:])
            sc_tiles[sb].append(s_t)

        # prologue MM2(0) with group prefetch, then MM1(1)
        for ti in range(TC):
            tg, sub = divmod(ti, SB // P)
            if sub == 0 and tg + 2 < NTG:
                issue_x1tg(tg + 2, tg + 2)
            emit_mm2_tile(0, ti)
        emit_mm1(1)
        maxbs = {}

        # ---------- softmax helpers ----------
        def emit_max_reduce(sb):
            """per-s max over partitions -> maxb [P,SB] broadcast tile."""
            mrow = p_row.tile([1, SB], FP32R, name=f"mrow{sb}", tag="mrow")
            trs, mcols = [], []
            for blk in range(SB // P):
                ps_tr = p_psum.tile([P, P], FP32R, name=f"ptr{sb}_{blk}", tag="ps")
                nc.tensor.transpose(
                    ps_tr[:], maxacc[sb][:, blk * P : (blk + 1) * P], identity[:]
                )
                trs.append(ps_tr)
            for blk in range(SB // P):
                mcol = p_row.tile([P, 1], FP32R, name=f"mcol{sb}_{blk}", tag=f"mcol{blk % 2}")
                nc.vector.reduce_max(mcol[:], trs[blk][:], axis=AX.X)
                mcols.append(mcol)
            ps_rrs = []
            for blk in range(SB // P):
                ps_rr = p_psum.tile([1, P], FP32R, name=f"prr{sb}_{blk}", tag="ps")
                nc.tensor.transpose(ps_rr[:], mcols[blk][:], identity[:])
                ps_rrs.append(ps_rr)
            for blk in range(SB // P):
                nc.vector.tensor_copy(mrow[:, blk * P : (blk + 1) * P], ps_rrs[blk][:])
            ps_mb = p_psum.tile([P, SB], FP32, name=f"pmb{sb}", tag="ps")
            nc.tensor.matmul(ps_mb[:], ones_row[:], mrow[:], start=True, stop=True)
            maxb = p_aux.tile([P, SB], FP32, name=f"maxb{sb}", tag="maxb")
            nc.vector.tensor_copy(maxb[:], ps_mb[:])
            return maxb

        def emit_exp_tile(sb, ti, maxb):
            if ti == 0:
                pb_tiles[sb] = []
            s_t = sc_tiles[sb][ti]
            nc.vector.tensor_sub(s_t[:], s_t[:], maxb[:])
            p_t = p_pb.tile([P, SB], BF16, name=f"pb{sb}_{ti}", tag="pb")
            nc.scalar.activation(p_t[:], s_t[:], AF.Exp, bias=0.0, scale=1.0)
            pb_tiles[sb].append(p_t)

        def emit_sum_mm(sb, ti, ps_sum):
            """accumulate softmax denominator on the PE: [1,SB] += 1^T @ pb."""
            nc.tensor.matmul(
                ps_sum[:], ones_colb[:], pb_tiles[sb][ti][:],
                start=(ti == 0), stop=(ti == TC - 1),
            )

        def emit_sum_recip(sb, ps_sum):
            rrow = p_row.tile([1, SB], FP32, name=f"rrow{sb}", tag="rrow")
            with nc.allow_low_precision(reason="softmax denom reciprocal; fp22 ok"):
                nc.vector.reciprocal_approx_fast(rrow[:], ps_sum[:])
            return rrow

        def emit_recip_bcast(sb, rrow):
            ps_rb = p_psum.tile([P, SB], FP32, name=f"prb{sb}", tag="ps")
            nc.tensor.matmul(ps_rb[:], ones_row32[:], rrow[:], start=True, stop=True)
            recipb = p_aux.tile([P, SB], FP32, name=f"recipb{sb}", tag="recipb")
            nc.vector.tensor_copy(recipb[:], ps_rb[:])
            return recipb

        def mm4_copy_out(sb, ps_list, ots, et0):
            for i, ps_o in enumerate(ps_list):
                o_t = p_ot.tile([P, SB], OT_DT, name=f"ot{sb}_{et0 + i}", tag="ot")
                nc.vector.tensor_copy(o_t[:], ps_o[:])
                ots.append(o_t)

        def emit_mm5(sb, ots, recipb):
            s0 = sb * SB
            for ft in range(FT):
                ps_f = p_psum.tile([P, SB], FP32, name=f"psf{sb}_{ft}", tag="ps")
                for ec in range(DC):
                    nc.tensor.matmul(
                        ps_f[:], fcw_sl(ec, ft), ots[ec][:],
                        start=(ec == 0), stop=(ec == DC - 1),
                    )
                tmp = p_tmp.tile([P, SB], FP32, name=f"tmp{sb}_{ft}", tag="tmp")
                nc.vector.tensor_mul(tmp[:], ps_f[:], recipb[:])
                o_out = p_oo.tile([P, SB], FP32, name=f"oo{sb}_{ft}", tag="oo")
                nc.scalar.activation(
                    o_out[:], tmp[:], AF.Relu,
                    bias=fcb_cols[:, ft : ft + 1], scale=1.0,
                )
                nc.sync.dma_start(outt_d[ft * P : (ft + 1) * P, s0 : s0 + SB], o_out[:])

        # ---------- steady-state blocks ----------
        maxbs[0] = emit_max_reduce(0)
        for sb in range(NSB):
            if sb + 2 < NSB:
                x2t_tiles[sb + 2] = issue_x2t(sb + 2)
            if sb + 1 < NSB:
                issue_x1tg((sb + 1) * NTG, 0)

            maxb = maxbs[sb]

            if sb + 1 < NSB:
                # PE chews MM2(sb+1) + denom matmuls while Scalar/DVE run exp(sb)
                ps_sum = p_psum1.tile([1, SB], FP32, name=f"psum{sb}", tag="ps1")
                for ti in range(TC):
                    emit_exp_tile(sb, ti, maxb)
                    tg, sub = divmod(ti, SB // P)
                    if sub == 0 and tg + 1 < NTG:
                        issue_x1tg((sb + 1) * NTG + tg + 1, tg + 1)
                    emit_mm2_tile(sb + 1, ti)
                    emit_sum_mm(sb, ti, ps_sum)
                rrow = emit_sum_recip(sb, ps_sum)
                if sb + 2 < NSB:
                    emit_mm1(sb + 2)
                # maxT(sb+1) here: the PE<->DVE ping-pong hides under MM4/MM5
                maxbs[sb + 1] = emit_max_reduce(sb + 1)
                ots = []
                for et in range(DC):
                    ps_o = p_psum.tile([P, SB], FP32, name=f"pso{sb}_{et}", tag="ps")
                    for ti in range(TC):
                        nc.tensor.matmul(
                            ps_o[:], x1_sl(ti, et), pb_tiles[sb][ti][:],
                            start=(ti == 0), stop=(ti == TC - 1),
                        )
                    mm4_copy_out(sb, [ps_o], ots, et)
                recipb = emit_recip_bcast(sb, rrow)
                emit_mm5(sb, ots, recipb)
            else:
                # last block: ti-outer MM4 on 7 held banks + denom bank; et7 after
                mm4_ps = [
                    p_psum1.tile([P, SB], FP32, name=f"pso{sb}_{et}", tag="ps1")
                    for et in range(4)
                ] + [
                    p_psum.tile([P, SB], FP32, name=f"pso{sb}_{et}", tag="ps")
                    for et in range(4, DC - 1)
                ]
                ps_sum = p_psum.tile([1, SB], FP32, name=f"psum{sb}", tag="ps")
                for ti in range(TC):
                    emit_exp_tile(sb, ti, maxb)
                    for et in range(DC - 1):
                        nc.tensor.matmul(
                            mm4_ps[et][:], x1_sl(ti, et), pb_tiles[sb][ti][:],
                            start=(ti == 0), stop=(ti == TC - 1),
                        )
                    emit_sum_mm(sb, ti, ps_sum)
                ots = []
                mm4_copy_out(sb, mm4_ps, ots, 0)
                rrow = emit_sum_recip(sb, ps_sum)
                ps_o7 = p_psum.tile([P, SB], FP32, name=f"pso{sb}_7", tag="ps")
                for ti in range(TC):
                    nc.tensor.matmul(
                        ps_o7[:], x1_sl(ti, DC - 1), pb_tiles[sb][ti][:],
                        start=(ti == 0), stop=(ti == TC - 1),
                    )
                mm4_copy_out(sb, [ps_o7], ots, DC - 1)
                recipb = emit_recip_bcast(sb, rrow)
                emit_mm5(sb, ots, recipb)

    nc.compile()
    return nc


_NC_CACHE = None


def _get_nc():
    global _NC_CACHE
    if _NC_CACHE is None:
        _NC_CACHE = build_nc()
    return _NC_CACHE


def make_in_maps(x1, x2, U, bias, fc_w, fc_b):
    import ml_dtypes

    x1 = np.ascontiguousarray(np.asarray(x1, dtype=np.float32))
    x2 = np.ascontiguousarray(np.asarray(x2, dtype=np.float32))
    U = np.ascontiguousarray(np.asarray(U, dtype=np.float32))
    bias = np.asarray(bias, dtype=np.float32)
    fc_w = np.asarray(fc_w, dtype=np.float32)
    fc_b = np.asarray(fc_b, dtype=np.float32)
    # relaid residents (same for every core)
    ug = np.ascontiguousarray(
        U.reshape(DC, P, D).transpose(1, 0, 2).reshape(P, DC * D)
    )
    fcwg = np.ascontiguousarray(
        fc_w.T.reshape(DC, P, F).transpose(1, 0, 2).reshape(P, DC * F)
    ).astype(ml_dtypes.bfloat16)
    biasg = np.ascontiguousarray(bias.reshape(DC, P).T)
    fcbg = np.ascontiguousarray(fc_b.reshape(FT, P).T)
    in_maps = []
    for b in range(B):
        x1t = x1[b].T  # [D, S]
        x1tg = np.ascontiguousarray(
            x1t.reshape(DC, P, NTG, SB).transpose(1, 2, 0, 3).reshape(P, NTG * DC * SB)
        )
        x1g = np.ascontiguousarray(
            x1[b].reshape(TC, P, D).transpose(1, 0, 2).reshape(P, TC * D)
        ).astype(ml_dtypes.bfloat16)
        in_maps.append(
            {
                "x1g": x1g,
                "x1tg": x1tg,
                "x2t": np.ascontiguousarray(x2[b].T),
                "ug": ug,
                "fcwg": fcwg,
                "biasg": biasg,
                "fcbg": fcbg,
            }
        )
    return in_maps


def kernel(x1, x2, U, bias, fc_w, fc_b):
    from concourse.bass_utils import run_bass_kernel_spmd

    nc = _get_nc()
    in_maps = make_in_maps(x1, x2, U, bias, fc_w, fc_b)
    res = run_bass_kernel_spmd(nc, in_maps, core_ids=list(range(B)))
    out = np.stack([np.ascontiguousarray(r["outt"].T) for r in res.results])
    return out.astype(np.float32)


# revision 34
# speedup vs baseline: 1.0377x; 1.0148x over previous
"""BiaffineAttn Trainium2 kernel.

Math (per batch b):
    t    = x2 @ U + bias[None, :]      [S, D]   (bias folded: x2 U x1^T + 1 (x1 bias)^T
                                                 == [x2|1] [[U];[bias^T]] x1^T)
    attn = t @ x1^T
    p    = softmax(attn, axis=-1)
    out  = relu((p @ x1) @ fc_w^T + fc_b)       [S, F]

Sharding: data-parallel over batch B=8, one batch per NeuronCore.

Per-core pipeline, transposed orientation (softmax key dim t' on partitions),
software-pipelined across S-superblocks so the PE never idles during softmax.
The interleave window overlays three instruction streams per t'-tile:
  PE:     MM2(sb+1) tile (8 mm, 4-buf PSUM ring) + one MM1(sb+2) dc-step
          (4 mm into 4 held PSUM banks)  -> 12 mm/tile keeps PE the pacer
  Scalar: exp(sb, tile) + MM2 psum->SBUF score copy
  DVE:    running max + softmax-denominator accumulation
  GpSimd: score - maxb subtract (SBUF-only; Pool cannot touch PSUM)
Block 2 has no MM1 to interleave, so half of MM4(2) (et 0-3, ti-outer) fills
the window; block 3 interleaves the full ti-outer MM4(3) in 8 held banks.

rowsum via single ones-column matmul; reciprocal_approx_fast; relu+bias via
Scalar activation; [F,SB] stores per superblock.

All tensors are host-relaid so every resident loads in ONE big DMA (32KB
rows) and each x1t t'-group is a single [128, 4096] transfer (2-deep ring).

Host side: builds the relaid views and transposes the [F,S] per-core output
back to [S,F] when gathering (fp32 DMA transpose does not exist on TRN2).
"""

import os
import sys
from contextlib import ExitStack

import numpy as np

for _p in ("/opt/trn_rl_repo", os.path.expanduser("~/.axon_site/_ro/trn_rl_repo")):
    if os.path.isdir(_p) and _p not in sys.path:
        sys.path.insert(0, _p)

import concourse.bass as bass
import concourse.mybir as mybir
import concourse.tile as tile
from concourse import bacc

B = 8
S = 2048          # sequence length (both s and t')
D = 1024          # d_model
F = 512           # fc output dim
P = 128
SB = 512          # s superblock (moving free dim of every matmul)
NSB = S // SB     # 4
DC = D // P       # 8 contraction chunks of d / e
TC = S // P       # 16 t' tiles
NTG = NSB         # 4 t' groups of 4 tiles
FT = F // P       # 4
FP32 = mybir.dt.float32
FP32R = mybir.dt.float32r
BF16 = mybir.dt.bfloat16
FP16 = mybir.dt.float16
AF = mybir.ActivationFunctionType
ALU = mybir.AluOpType
AX = mybir.AxisListType

OT_DT = BF16      # dtype of MM4 output tiles (MM5 moving operand)


def build_nc():
    nc = bacc.Bacc(
        "TRN2",
        target_bir_lowering=False,
        debug=False,
        enable_asserts=False,
    )

    # host-relaid tensors: row p holds the p-th partition's data for every tile
    x1_d = nc.dram_tensor("x1g", [P, TC * D], BF16, kind="ExternalInput")
    x1t_d = nc.dram_tensor("x1tg", [P, NTG * DC * SB], FP16, kind="ExternalInput")
    x2t_d = nc.dram_tensor("x2t", [D, S], FP32R, kind="ExternalInput")
    u_d = nc.dram_tensor("ug", [P, DC * D], FP32R, kind="ExternalInput")
    fcwt_d = nc.dram_tensor("fcwg", [P, DC * F], BF16, kind="ExternalInput")
    bias_d = nc.dram_tensor("biasg", [P, DC], FP32, kind="ExternalInput")
    fcb_d = nc.dram_tensor("fcbg", [P, FT], FP32, kind="ExternalInput")
    outt_d = nc.dram_tensor("outt", [F, S], FP32, kind="ExternalOutput")

    with tile.TileContext(nc) as tc, ExitStack() as ctx:
        # ---------- pools ----------
        p_u = ctx.enter_context(tc.tile_pool(name="ures", bufs=1))
        p_x1 = ctx.enter_context(tc.tile_pool(name="x1res", bufs=1))
        p_bc = ctx.enter_context(tc.tile_pool(name="biascols", bufs=1))
        p_fcb = ctx.enter_context(tc.tile_pool(name="fcbcols", bufs=1))
        p_fcw = ctx.enter_context(tc.tile_pool(name="fcwres", bufs=1))
        p_ones = ctx.enter_context(tc.tile_pool(name="ones", bufs=1))
        p_psum = ctx.enter_context(tc.tile_pool(name="psum", bufs=4, space="PSUM"))
        p_psum1 = ctx.enter_context(tc.tile_pool(name="psum1", bufs=4, space="PSUM"))
        p_x2t = ctx.enter_context(tc.tile_pool(name="x2ts", bufs=5))
        p_x1tc = ctx.enter_context(tc.tile_pool(name="x1tgs", bufs=2))
        p_tt = ctx.enter_context(tc.tile_pool(name="tts", bufs=9))
        p_sc = ctx.enter_context(tc.tile_pool(name="scores", bufs=TC))
        p_pb = ctx.enter_context(tc.tile_pool(name="pbf", bufs=TC))
        p_ot = ctx.enter_context(tc.tile_pool(name="ots", bufs=DC))
        p_aux = ctx.enter_context(tc.tile_pool(name="aux", bufs=1))
        p_row = ctx.enter_context(tc.tile_pool(name="rows", bufs=1))
        p_tmp = ctx.enter_context(tc.tile_pool(name="tmps", bufs=1))
        p_oo = ctx.enter_context(tc.tile_pool(name="oos", bufs=1))

        # ---------- prologue DMAs: MM1(0) critical path first ----------
        u_big = p_u.tile([P, DC * D], FP32R, name="ug", tag="ur")
        for q in range(4):
            nc.sync.dma_start(
                u_big[:, q * 2 * D : (q + 1) * 2 * D],
                u_d[:, q * 2 * D : (q + 1) * 2 * D],
            )

        def u_sl(dc, et):
            return u_big[:, dc * D + et * P : dc * D + (et + 1) * P]

        # prologue x2t on the Scalar HWDGE queue: overlaps with u on sync
        x2t_tiles = {0: []}
        for dc in range(DC):
            x2_t = p_x2t.tile([P, SB], FP32R, name=f"x2t0_{dc}", tag="x2t")
            nc.scalar.dma_start(x2_t[:], x2t_d[dc * P : (dc + 1) * P, 0:SB])
            x2t_tiles[0].append(x2_t)

        # x1t t'-group streaming (MM2 stationaries): one DMA per group, ring 2
        x1tg = {}

        def issue_x1tg(key, tg):
            # gpsimd queue: ring-WAR blocked group loads must not head-of-line
            # block the sync queue
            t = p_x1tc.tile([P, DC * SB], FP16, name=f"x1tg{key}", tag="x1tg")
            nc.gpsimd.dma_start(t[:], x1t_d[:, tg * DC * SB : (tg + 1) * DC * SB])
            x1tg[key] = t

        issue_x1tg(0, 0)
        issue_x1tg(1, 1)
        bias_cols = p_bc.tile([P, DC], FP32, name="bc", tag="bc")
        nc.scalar.dma_start(bias_cols[:], bias_d[:, :])

        identity32 = p_ones.tile([P, P], FP32, name="ident32", tag="ident32")
        nc.gpsimd.memset(identity32[:], 0.0)
        nc.gpsimd.affine_select(
            out=identity32[:], in_=identity32[:], compare_op=ALU.not_equal,
            fill=1.0, base=0, pattern=[[-1, P]], channel_multiplier=1,
        )
        identity = p_ones.tile([P, P], FP32R, name="ident", tag="ident")
        nc.scalar.activation(identity[:], identity32[:], AF.Identity, bias=0.0, scale=1.0)
        ones_row = p_ones.tile([1, P], FP32R, name="ones_row", tag="ones_row")
        nc.scalar.activation(ones_row[:], identity[0:1, :], AF.Identity, bias=1.0, scale=0.0)
        ones_colb = p_ones.tile([P, 1], BF16, name="ones_colb", tag="ones_colb")
        nc.scalar.activation(ones_colb[:], identity[:, 0:1], AF.Identity, bias=1.0, scale=0.0)
        ones_row32 = p_ones.tile([1, P], FP32, name="ones_row32", tag="ones_row32")
        nc.scalar.activation(ones_row32[:], identity[0:1, :], AF.Identity, bias=1.0, scale=0.0)

        def issue_x2t(sb, eng=None):
            eng = eng or nc.sync
            tiles = []
            for dc in range(DC):
                x2_t = p_x2t.tile([P, SB], FP32R, name=f"x2t{sb}_{dc}", tag="x2t")
                eng.dma_start(
                    x2_t[:], x2t_d[dc * P : (dc + 1) * P, sb * SB : (sb + 1) * SB]
                )
                tiles.append(x2_t)
            return tiles

        x2t_tiles[1] = issue_x2t(1, nc.scalar)

        # residents (needed from MM4(0)/MM5(0) on) issued after all critical
        # prologue streams
        x1_big = p_x1.tile([P, TC * D], BF16, name="x1g", tag="x1r")
        nc.sync.dma_start(x1_big[:, : TC * D // 2], x1_d[:, : TC * D // 2])
        nc.sync.dma_start(x1_big[:, TC * D // 2 :], x1_d[:, TC * D // 2 :])

        def x1_sl(ti, et):
            return x1_big[:, ti * D + et * P : ti * D + (et + 1) * P]

        fcw_big = p_fcw.tile([P, DC * F], BF16, name="fcwg", tag="fcw")
        nc.sync.dma_start(fcw_big[:], fcwt_d[:, :])

        def fcw_sl(ec, ft):
            return fcw_big[:, ec * F + ft * P : ec * F + (ft + 1) * P]

        fcb_cols = p_fcb.tile([P, FT], FP32, name="fcb", tag="fcb")
        nc.sync.dma_start(fcb_cols[:], fcb_d[:, :])

        # ---------- MM1: ttT = (x2 @ U)^T + bias, 8 held banks (both pools) ----------
        tt = {}

        def emit_mm1(sb):
            tt[sb] = []
            ps = [
                p_psum1.tile([P, SB], FP32, name=f"ps1_{sb}_{et}", tag="ps1")
                for et in range(4)
            ] + [
                p_psum.tile([P, SB], FP32, name=f"ps1b_{sb}_{et}", tag="ps")
                for et in range(4, DC)
            ]
            for dc in range(DC):
                for et in range(DC):
                    nc.tensor.matmul(
                        ps[et][:], u_sl(dc, et), x2t_tiles[sb][dc][:],
                        start=(dc == 0), stop=(dc == DC - 1),
                    )
            for et in range(DC):
                t_t = p_tt.tile([P, SB], FP16, name=f"tt{sb}_{et}", tag="tt")
                nc.scalar.activation(
                    t_t[:], ps[et][:], AF.Identity,
                    bias=bias_cols[:, et : et + 1], scale=1.0,
                )
                tt[sb].append(t_t)

        emit_mm1(0)

        # ---------- MM2 per-tile emitter: scoresT tile + running max ----------
        sc_tiles = {}
        maxacc = {}
        sumacc = {}
        pb_tiles = {}

        def emit_mm2_tile(sb, ti):
            tg, sub = divmod(ti, SB // P)
            if ti == 0:
                sc_tiles[sb] = []
                maxacc[sb] = p_aux.tile(
                    [P, SB], FP32R, name=f"maxacc{sb}", tag="maxacc"
                )
            ps_s = p_psum.tile([P, SB], FP32, name=f"pss{sb}_{ti}", tag="ps")
            grp = x1tg[sb * NTG + tg]
            for ec in range(DC):
                nc.tensor.matmul(
                    ps_s[:],
                    grp[:, ec * SB + sub * P : ec * SB + (sub + 1) * P],
                    tt[sb][ec][:],
                    start=(ec == 0), stop=(ec == DC - 1),
                )
            s_t = p_sc.tile([P, SB], FP32, name=f"sc{sb}_{ti}", tag="sc")
            nc.scalar.copy(s_t[:], ps_s[:])
            if ti == 0:
                nc.vector.tensor_copy(maxacc[sb][:], s_t[:])
            else:
                nc.vector.tensor_max(maxacc[sb][:], maxacc[sb][:], s_t[:])
            sc_tiles[sb].append(s_t)

        # prologue MM2(0) with group prefetch, then MM1(1)
        for ti in range(TC):
            tg, sub = divmod(ti, SB // P)
            if sub == 0 and tg + 2 < NTG:
                issue_x1tg(tg + 2, tg + 2)
            emit_mm2_tile(0, ti)
        emit_mm1(1)
        maxbs = {}

        # ---------- softmax helpers ----------
        def emit_max_reduce(sb):
            """per-s max over partitions -> maxb [P,SB] broadcast tile."""
            mrow = p_row.tile([1, SB], FP32R, name=f"mrow{sb}", tag="mrow")
            trs, mcols = [], []
            for blk in range(SB // P):
                ps_tr = p_psum.tile([P, P], FP32R, name=f"ptr{sb}_{blk}", tag="ps")
                nc.tensor.transpose(
                    ps_tr[:], maxacc[sb][:, blk * P : (blk + 1) * P], identity[:]
                )
                trs.append(ps_tr)
            for blk in range(SB // P):
                mcol = p_row.tile([P, 1], FP32R, name=f"mcol{sb}_{blk}", tag=f"mcol{blk % 2}")
                nc.vector.reduce_max(mcol[:], trs[blk][:], axis=AX.X)
                mcols.append(mcol)
            ps_rrs = []
            for blk in range(SB // P):
                ps_rr = p_psum.tile([1, P], FP32R, name=f"prr{sb}_{blk}", tag="ps")
                nc.tensor.transpose(ps_rr[:], mcols[blk][:], identity[:])
                ps_rrs.append(ps_rr)
            for blk in range(SB // P):
                nc.vector.tensor_copy(mrow[:, blk * P : (blk + 1) * P], ps_rrs[blk][:])
            ps_mb = p_psum.tile([P, SB], FP32, name=f"pmb{sb}", tag="ps")
            nc.tensor.matmul(ps_mb[:], ones_row[:], mrow[:], start=True, stop=True)
            maxb = p_aux.tile([P, SB], FP32, name=f"maxb{sb}", tag="maxb")
            nc.vector.tensor_copy(maxb[:], ps_mb[:])
            return maxb

        def emit_exp_tile(sb, ti, maxb):
            if ti == 0:
                pb_tiles[sb] = []
            s_t = sc_tiles[sb][ti]
            nc.vector.tensor_sub(s_t[:], s_t[:], maxb[:])
            p_t = p_pb.tile([P, SB], BF16, name=f"pb{sb}_{ti}", tag="pb")
            nc.scalar.activation(p_t[:], s_t[:], AF.Exp, bias=0.0, scale=1.0)
            pb_tiles[sb].append(p_t)

        def emit_sum_mm(sb, ti, ps_sum):
            """accumulate softmax denominator on the PE: [1,SB] += 1^T @ pb."""
            nc.tensor.matmul(
                ps_sum[:], ones_colb[:], pb_tiles[sb][ti][:],
                start=(ti == 0), stop=(ti == TC - 1),
            )

        def emit_sum_recip(sb, ps_sum):
            rrow = p_row.tile([1, SB], FP32, name=f"rrow{sb}", tag="rrow")
            with nc.allow_low_precision(reason="softmax denom reciprocal; fp22 ok"):
                nc.vector.reciprocal_approx_fast(rrow[:], ps_sum[:])
            return rrow

        def emit_recip_bcast(sb, rrow):
            ps_rb = p_psum.tile([P, SB], FP32, name=f"prb{sb}", tag="ps")
            nc.tensor.matmul(ps_rb[:], ones_row32[:], rrow[:], start=True, stop=True)
            recipb = p_aux.tile([P, SB], FP32, name=f"recipb{sb}", tag="recipb")
            nc.vector.tensor_copy(recipb[:], ps_rb[:])
            return recipb

        def mm4_copy_out(sb, ps_list, ots, et0):
            for i, ps_o in enumerate(ps_list):
                o_t = p_ot.tile([P, SB], OT_DT, name=f"ot{sb}_{et0 + i}", tag="ot")
                nc.vector.tensor_copy(o_t[:], ps_o[:])
                ots.append(o_t)

        def emit_mm5(sb, ots, recipb):
            s0 = sb * SB
            for ft in range(FT):
                ps_f = p_psum.tile([P, SB], FP32, name=f"psf{sb}_{ft}", tag="ps")
                for ec in range(DC):
                    nc.tensor.matmul(
                        ps_f[:], fcw_sl(ec, ft), ots[ec][:],
                        start=(ec == 0), stop=(ec == DC - 1),
                    )
                tmp = p_tmp.tile([P, SB], FP32, name=f"tmp{sb}_{ft}", tag="tmp")
                nc.vector.tensor_mul(tmp[:], ps_f[:], recipb[:])
                o_out = p_oo.tile([P, SB], FP32, name=f"oo{sb}_{ft}", tag="oo")
                nc.scalar.activation(
                    o_out[:], tmp[:], AF.Relu,
                    bias=fcb_cols[:, ft : ft + 1], scale=1.0,
                )
                nc.sync.dma_start(outt_d[ft * P : (ft + 1) * P, s0 : s0 + SB], o_out[:])

        # ---------- steady-state blocks ----------
        maxbs[0] = emit_max_reduce(0)
        for sb in range(NSB):
            if sb + 2 < NSB:
                x2t_tiles[sb + 2] = issue_x2t(sb + 2)
            if sb + 1 < NSB:
                issue_x1tg((sb + 1) * NTG, 0)

            maxb = maxbs[sb]

            if sb + 1 < NSB:
                # PE chews MM2(sb+1) + denom matmuls while Scalar/DVE run exp(sb)
                ps_sum = p_psum1.tile([1, SB], FP32, name=f"psum{sb}", tag="ps1")
                for ti in range(TC):
                    emit_exp_tile(sb, ti, maxb)
                    tg, sub = divmod(ti, SB // P)
                    if sub == 0 and tg + 1 < NTG:
                        issue_x1tg((sb + 1) * NTG + tg + 1, tg + 1)
                    emit_mm2_tile(sb + 1, ti)
                    emit_sum_mm(sb, ti, ps_sum)
                rrow = emit_sum_recip(sb, ps_sum)
                if sb + 2 < NSB:
                    emit_mm1(sb + 2)
                # maxT(sb+1) here: the PE<->DVE ping-pong hides under MM4/MM5
                maxbs[sb + 1] = emit_max_reduce(sb + 1)
                ots = []
                for et in range(DC):
                    ps_o = p_psum.tile([P, SB], FP32, name=f"pso{sb}_{et}", tag="ps")
                    for ti in range(TC):
                        nc.tensor.matmul(
                            ps_o[:], x1_sl(ti, et), pb_tiles[sb][ti][:],
                            start=(ti == 0), stop=(ti == TC - 1),
                        )
                    mm4_copy_out(sb, [ps_o], ots, et)
                recipb = emit_recip_bcast(sb, rrow)
                emit_mm5(sb, ots, recipb)
            else:
                # last block: ti-outer MM4 on 7 held banks + denom bank; et7 after
                mm4_ps = [
                    p_psum1.tile([P, SB], FP32, name=f"pso{sb}_{et}", tag="ps1")
                    for et in range(4)
                ] + [
                    p_psum.tile([P, SB], FP32, name=f"pso{sb}_{et}", tag="ps")
                    for et in range(4, DC - 1)
                ]
                ps_sum = p_psum.tile([1, SB], FP32, name=f"psum{sb}", tag="ps")
                for ti in range(TC):
                    emit_exp_tile(sb, ti, maxb)
                    for et in range(DC - 1):
                        nc.tensor.matmul(
                            mm4_ps[et][:], x1_sl(ti, et), pb_tiles[sb][ti][:],
                            start=(ti == 0), stop=(ti == TC - 1),
                        )
                    emit_sum_mm(sb, ti, ps_sum)
                ots = []
                mm4_copy_out(sb, mm4_ps, ots, 0)
                rrow = emit_sum_recip(sb, ps_sum)
                ps_o7 = p_psum.tile([P, SB], FP32, name=f"pso{sb}_7", tag="ps")
                for ti in range(TC):
                    nc.tensor.matmul(
                        ps_o7[:], x1_sl(ti, DC - 1), pb_tiles[sb][ti][:],
                        start=(ti == 0), stop=(ti == TC - 1),
                    )
                mm4_copy_out(sb, [ps_o7], ots, DC - 1)
                recipb = emit_recip_bcast(sb, rrow)
                emit_mm5(sb, ots, recipb)

    nc.compile()
    return nc


_NC_CACHE = None


def _get_nc():
    global _NC_CACHE
    if _NC_CACHE is None:
        _NC_CACHE = build_nc()
    return _NC_CACHE


def make_in_maps(x1, x2, U, bias, fc_w, fc_b):
    import ml_dtypes

    x1 = np.ascontiguousarray(np.asarray(x1, dtype=np.float32))
    x2 = np.ascontiguousarray(np.asarray(x2, dtype=np.float32))
    U = np.ascontiguousarray(np.asarray(U, dtype=np.float32))
    bias = np.asarray(bias, dtype=np.float32)
    fc_w = np.asarray(fc_w, dtype=np.float32)
    fc_b = np.asarray(fc_b, dtype=np.float32)
    # relaid residents (same for every core)
    ug = np.ascontiguousarray(
        U.reshape(DC, P, D).transpose(1, 0, 2).reshape(P, DC * D)
    )
    fcwg = np.ascontiguousarray(
        fc_w.T.reshape(DC, P, F).transpose(1, 0, 2).reshape(P, DC * F)
    ).astype(ml_dtypes.bfloat16)
    biasg = np.ascontiguousarray(bias.reshape(DC, P).T)
    fcbg = np.ascontiguousarray(fc_b.reshape(FT, P).T)
    in_maps = []
    for b in range(B):
        x1t = x1[b].T  # [D, S]
        x1tg = np.ascontiguousarray(
            x1t.reshape(DC, P, NTG, SB).transpose(1, 2, 0, 3).reshape(P, NTG * DC * SB)
        ).astype(np.float16)
        x1g = np.ascontiguousarray(
            x1[b].reshape(TC, P, D).transpose(1, 0, 2).reshape(P, TC * D)
        ).astype(ml_dtypes.bfloat16)
        in_maps.append(
            {
                "x1g": x1g,
                "x1tg": x1tg,
                "x2t": np.ascontiguousarray(x2[b].T),
                "ug": ug,
                "fcwg": fcwg,
                "biasg": biasg,
                "fcbg": fcbg,
            }
        )
    return in_maps


def kernel(x1, x2, U, bias, fc_w, fc_b):
    from concourse.bass_utils import run_bass_kernel_spmd

    nc = _get_nc()
    in_maps = make_in_maps(x1, x2, U, bias, fc_w, fc_b)
    res = run_bass_kernel_spmd(nc, in_maps, core_ids=list(range(B)))
    out = np.stack([np.ascontiguousarray(r["outt"].T) for r in res.results])
    return out.astype(np.float32)


# revision 35
# speedup vs baseline: 1.0789x; 1.0397x over previous
"""BiaffineAttn Trainium2 kernel.

Math (per batch b):
    t    = x2 @ U + bias[None, :]      [S, D]   (bias folded: x2 U x1^T + 1 (x1 bias)^T
                                                 == [x2|1] [[U];[bias^T]] x1^T)
    attn = t @ x1^T
    p    = softmax(attn, axis=-1)
    out  = relu((p @ x1) @ fc_w^T + fc_b)       [S, F]

Sharding: data-parallel over batch B=8, one batch per NeuronCore.

Per-core pipeline, transposed orientation (softmax key dim t' on partitions),
software-pipelined across S-superblocks so the PE never idles during softmax.
The interleave window overlays three instruction streams per t'-tile:
  PE:     MM2(sb+1) tile (8 mm, 4-buf PSUM ring) + one MM1(sb+2) dc-step
          (4 mm into 4 held PSUM banks)  -> 12 mm/tile keeps PE the pacer
  Scalar: exp(sb, tile) + MM2 psum->SBUF score copy
  DVE:    running max + softmax-denominator accumulation
  GpSimd: score - maxb subtract (SBUF-only; Pool cannot touch PSUM)
Block 2 has no MM1 to interleave, so half of MM4(2) (et 0-3, ti-outer) fills
the window; block 3 interleaves the full ti-outer MM4(3) in 8 held banks.

rowsum via single ones-column matmul; reciprocal_approx_fast; relu+bias via
Scalar activation; [F,SB] stores per superblock.

All tensors are host-relaid so every resident loads in ONE big DMA (32KB
rows) and each x1t t'-group is a single [128, 4096] transfer (2-deep ring).

Host side: builds the relaid views and transposes the [F,S] per-core output
back to [S,F] when gathering (fp32 DMA transpose does not exist on TRN2).
"""

import os
import sys
from contextlib import ExitStack

import numpy as np

for _p in ("/opt/trn_rl_repo", os.path.expanduser("~/.axon_site/_ro/trn_rl_repo")):
    if os.path.isdir(_p) and _p not in sys.path:
        sys.path.insert(0, _p)

import concourse.bass as bass
import concourse.mybir as mybir
import concourse.tile as tile
from concourse import bacc

B = 8
S = 2048          # sequence length (both s and t')
D = 1024          # d_model
F = 512           # fc output dim
P = 128
SB = 512          # s superblock (moving free dim of every matmul)
NSB = S // SB     # 4
DC = D // P       # 8 contraction chunks of d / e
TC = S // P       # 16 t' tiles
NTG = NSB         # 4 t' groups of 4 tiles
FT = F // P       # 4
FP32 = mybir.dt.float32
FP32R = mybir.dt.float32r
BF16 = mybir.dt.bfloat16
FP16 = mybir.dt.float16
AF = mybir.ActivationFunctionType
ALU = mybir.AluOpType
AX = mybir.AxisListType

OT_DT = BF16      # dtype of MM4 output tiles (MM5 moving operand)


def build_nc():
    nc = bacc.Bacc(
        "TRN2",
        target_bir_lowering=False,
        debug=False,
        enable_asserts=False,
    )

    # host-relaid tensors: row p holds the p-th partition's data for every tile
    x1_d = nc.dram_tensor("x1g", [P, TC * D], BF16, kind="ExternalInput")
    x1t_d = nc.dram_tensor("x1tg", [P, NTG * DC * SB], FP16, kind="ExternalInput")
    x2t_d = nc.dram_tensor("x2t", [D, S], FP16, kind="ExternalInput")
    u_d = nc.dram_tensor("ug", [P, DC * D], FP16, kind="ExternalInput")
    fcwt_d = nc.dram_tensor("fcwg", [P, DC * F], BF16, kind="ExternalInput")
    bias_d = nc.dram_tensor("biasg", [P, DC], FP32, kind="ExternalInput")
    fcb_d = nc.dram_tensor("fcbg", [P, FT], FP32, kind="ExternalInput")
    outt_d = nc.dram_tensor("outt", [F, S], FP32, kind="ExternalOutput")

    with tile.TileContext(nc) as tc, ExitStack() as ctx:
        # ---------- pools ----------
        p_u = ctx.enter_context(tc.tile_pool(name="ures", bufs=1))
        p_x1 = ctx.enter_context(tc.tile_pool(name="x1res", bufs=1))
        p_bc = ctx.enter_context(tc.tile_pool(name="biascols", bufs=1))
        p_fcb = ctx.enter_context(tc.tile_pool(name="fcbcols", bufs=1))
        p_fcw = ctx.enter_context(tc.tile_pool(name="fcwres", bufs=1))
        p_ones = ctx.enter_context(tc.tile_pool(name="ones", bufs=1))
        p_psum = ctx.enter_context(tc.tile_pool(name="psum", bufs=4, space="PSUM"))
        p_psum1 = ctx.enter_context(tc.tile_pool(name="psum1", bufs=4, space="PSUM"))
        p_x2t = ctx.enter_context(tc.tile_pool(name="x2ts", bufs=8))
        p_x1tc = ctx.enter_context(tc.tile_pool(name="x1tgs", bufs=4))
        p_tt = ctx.enter_context(tc.tile_pool(name="tts", bufs=9))
        p_sc = ctx.enter_context(tc.tile_pool(name="scores", bufs=TC))
        p_pb = ctx.enter_context(tc.tile_pool(name="pbf", bufs=TC))
        p_ot = ctx.enter_context(tc.tile_pool(name="ots", bufs=DC))
        p_aux = ctx.enter_context(tc.tile_pool(name="aux", bufs=1))
        p_row = ctx.enter_context(tc.tile_pool(name="rows", bufs=1))
        p_tmp = ctx.enter_context(tc.tile_pool(name="tmps", bufs=1))
        p_oo = ctx.enter_context(tc.tile_pool(name="oos", bufs=1))

        # ---------- prologue DMAs: MM1(0) critical path first ----------
        u_big = p_u.tile([P, DC * D], FP16, name="ug", tag="ur")
        for q in range(4):
            nc.sync.dma_start(
                u_big[:, q * 2 * D : (q + 1) * 2 * D],
                u_d[:, q * 2 * D : (q + 1) * 2 * D],
            )

        def u_sl(dc, et):
            return u_big[:, dc * D + et * P : dc * D + (et + 1) * P]

        # prologue x2t on the Scalar HWDGE queue: overlaps with u on sync
        x2t_tiles = {0: []}
        for dc in range(DC):
            x2_t = p_x2t.tile([P, SB], FP16, name=f"x2t0_{dc}", tag="x2t")
            nc.scalar.dma_start(x2_t[:], x2t_d[dc * P : (dc + 1) * P, 0:SB])
            x2t_tiles[0].append(x2_t)

        # x1t t'-group streaming (MM2 stationaries): one DMA per group, ring 2
        x1tg = {}

        def issue_x1tg(key, tg):
            # gpsimd queue: ring-WAR blocked group loads must not head-of-line
            # block the sync queue
            t = p_x1tc.tile([P, DC * SB], FP16, name=f"x1tg{key}", tag="x1tg")
            nc.gpsimd.dma_start(t[:], x1t_d[:, tg * DC * SB : (tg + 1) * DC * SB])
            x1tg[key] = t

        issue_x1tg(0, 0)
        issue_x1tg(1, 1)
        bias_cols = p_bc.tile([P, DC], FP32, name="bc", tag="bc")
        nc.scalar.dma_start(bias_cols[:], bias_d[:, :])

        identity32 = p_ones.tile([P, P], FP32, name="ident32", tag="ident32")
        nc.gpsimd.memset(identity32[:], 0.0)
        nc.gpsimd.affine_select(
            out=identity32[:], in_=identity32[:], compare_op=ALU.not_equal,
            fill=1.0, base=0, pattern=[[-1, P]], channel_multiplier=1,
        )
        identity = p_ones.tile([P, P], FP32R, name="ident", tag="ident")
        nc.scalar.activation(identity[:], identity32[:], AF.Identity, bias=0.0, scale=1.0)
        ones_row = p_ones.tile([1, P], FP32R, name="ones_row", tag="ones_row")
        nc.scalar.activation(ones_row[:], identity[0:1, :], AF.Identity, bias=1.0, scale=0.0)
        ones_colb = p_ones.tile([P, 1], BF16, name="ones_colb", tag="ones_colb")
        nc.scalar.activation(ones_colb[:], identity[:, 0:1], AF.Identity, bias=1.0, scale=0.0)
        ones_row32 = p_ones.tile([1, P], FP32, name="ones_row32", tag="ones_row32")
        nc.scalar.activation(ones_row32[:], identity[0:1, :], AF.Identity, bias=1.0, scale=0.0)

        def issue_x2t(sb, eng=None):
            eng = eng or nc.sync
            tiles = []
            for dc in range(DC):
                x2_t = p_x2t.tile([P, SB], FP16, name=f"x2t{sb}_{dc}", tag="x2t")
                eng.dma_start(
                    x2_t[:], x2t_d[dc * P : (dc + 1) * P, sb * SB : (sb + 1) * SB]
                )
                tiles.append(x2_t)
            return tiles

        x2t_tiles[1] = issue_x2t(1, nc.scalar)

        # residents (needed from MM4(0)/MM5(0) on) issued after all critical
        # prologue streams
        x1_big = p_x1.tile([P, TC * D], BF16, name="x1g", tag="x1r")
        nc.sync.dma_start(x1_big[:, : TC * D // 2], x1_d[:, : TC * D // 2])
        nc.sync.dma_start(x1_big[:, TC * D // 2 :], x1_d[:, TC * D // 2 :])

        def x1_sl(ti, et):
            return x1_big[:, ti * D + et * P : ti * D + (et + 1) * P]

        fcw_big = p_fcw.tile([P, DC * F], BF16, name="fcwg", tag="fcw")
        nc.sync.dma_start(fcw_big[:], fcwt_d[:, :])

        def fcw_sl(ec, ft):
            return fcw_big[:, ec * F + ft * P : ec * F + (ft + 1) * P]

        fcb_cols = p_fcb.tile([P, FT], FP32, name="fcb", tag="fcb")
        nc.sync.dma_start(fcb_cols[:], fcb_d[:, :])

        # ---------- MM1: ttT = (x2 @ U)^T + bias, 8 held banks (both pools) ----------
        tt = {}

        def emit_mm1(sb):
            tt[sb] = []
            ps = [
                p_psum1.tile([P, SB], FP32, name=f"ps1_{sb}_{et}", tag="ps1")
                for et in range(4)
            ] + [
                p_psum.tile([P, SB], FP32, name=f"ps1b_{sb}_{et}", tag="ps")
                for et in range(4, DC)
            ]
            for dc in range(DC):
                for et in range(DC):
                    nc.tensor.matmul(
                        ps[et][:], u_sl(dc, et), x2t_tiles[sb][dc][:],
                        start=(dc == 0), stop=(dc == DC - 1),
                    )
            for et in range(DC):
                t_t = p_tt.tile([P, SB], FP16, name=f"tt{sb}_{et}", tag="tt")
                nc.scalar.activation(
                    t_t[:], ps[et][:], AF.Identity,
                    bias=bias_cols[:, et : et + 1], scale=1.0,
                )
                tt[sb].append(t_t)

        emit_mm1(0)

        # ---------- MM2 per-tile emitter: scoresT tile + running max ----------
        sc_tiles = {}
        maxacc = {}
        sumacc = {}
        pb_tiles = {}

        def emit_mm2_tile(sb, ti):
            tg, sub = divmod(ti, SB // P)
            if ti == 0:
                sc_tiles[sb] = []
                maxacc[sb] = p_aux.tile(
                    [P, SB], FP32R, name=f"maxacc{sb}", tag="maxacc"
                )
            ps_s = p_psum.tile([P, SB], FP32, name=f"pss{sb}_{ti}", tag="ps")
            grp = x1tg[sb * NTG + tg]
            for ec in range(DC):
                nc.tensor.matmul(
                    ps_s[:],
                    grp[:, ec * SB + sub * P : ec * SB + (sub + 1) * P],
                    tt[sb][ec][:],
                    start=(ec == 0), stop=(ec == DC - 1),
                )
            s_t = p_sc.tile([P, SB], FP32, name=f"sc{sb}_{ti}", tag="sc")
            nc.scalar.copy(s_t[:], ps_s[:])
            if ti == 0:
                nc.vector.tensor_copy(maxacc[sb][:], s_t[:])
            else:
                nc.vector.tensor_max(maxacc[sb][:], maxacc[sb][:], s_t[:])
            sc_tiles[sb].append(s_t)

        # prologue MM2(0) with group prefetch, then MM1(1)
        for ti in range(TC):
            tg, sub = divmod(ti, SB // P)
            if sub == 0 and tg + 2 < NTG:
                issue_x1tg(tg + 2, tg + 2)
            emit_mm2_tile(0, ti)
        emit_mm1(1)
        maxbs = {}

        # ---------- softmax helpers ----------
        def emit_max_reduce(sb):
            """per-s max over partitions -> maxb [P,SB] broadcast tile."""
            mrow = p_row.tile([1, SB], FP32R, name=f"mrow{sb}", tag="mrow")
            trs, mcols = [], []
            for blk in range(SB // P):
                ps_tr = p_psum.tile([P, P], FP32R, name=f"ptr{sb}_{blk}", tag="ps")
                nc.tensor.transpose(
                    ps_tr[:], maxacc[sb][:, blk * P : (blk + 1) * P], identity[:]
                )
                trs.append(ps_tr)
            for blk in range(SB // P):
                mcol = p_row.tile([P, 1], FP32R, name=f"mcol{sb}_{blk}", tag=f"mcol{blk % 2}")
                nc.vector.reduce_max(mcol[:], trs[blk][:], axis=AX.X)
                mcols.append(mcol)
            ps_rrs = []
            for blk in range(SB // P):
                ps_rr = p_psum.tile([1, P], FP32R, name=f"prr{sb}_{blk}", tag="ps")
                nc.tensor.transpose(ps_rr[:], mcols[blk][:], identity[:])
                ps_rrs.append(ps_rr)
            for blk in range(SB // P):
                nc.vector.tensor_copy(mrow[:, blk * P : (blk + 1) * P], ps_rrs[blk][:])
            ps_mb = p_psum.tile([P, SB], FP32, name=f"pmb{sb}", tag="ps")
            nc.tensor.matmul(ps_mb[:], ones_row[:], mrow[:], start=True, stop=True)
            maxb = p_aux.tile([P, SB], FP32, name=f"maxb{sb}", tag="maxb")
            nc.vector.tensor_copy(maxb[:], ps_mb[:])
            return maxb

        def emit_exp_tile(sb, ti, maxb):
            if ti == 0:
                pb_tiles[sb] = []
            s_t = sc_tiles[sb][ti]
            nc.vector.tensor_sub(s_t[:], s_t[:], maxb[:])
            p_t = p_pb.tile([P, SB], BF16, name=f"pb{sb}_{ti}", tag="pb")
            nc.scalar.activation(p_t[:], s_t[:], AF.Exp, bias=0.0, scale=1.0)
            pb_tiles[sb].append(p_t)

        def emit_sum_mm(sb, ti, ps_sum):
            """accumulate softmax denominator on the PE: [1,SB] += 1^T @ pb."""
            nc.tensor.matmul(
                ps_sum[:], ones_colb[:], pb_tiles[sb][ti][:],
                start=(ti == 0), stop=(ti == TC - 1),
            )

        def emit_sum_recip(sb, ps_sum):
            rrow = p_row.tile([1, SB], FP32, name=f"rrow{sb}", tag="rrow")
            with nc.allow_low_precision(reason="softmax denom reciprocal; fp22 ok"):
                nc.vector.reciprocal_approx_fast(rrow[:], ps_sum[:])
            return rrow

        def emit_recip_bcast(sb, rrow):
            ps_rb = p_psum.tile([P, SB], FP32, name=f"prb{sb}", tag="ps")
            nc.tensor.matmul(ps_rb[:], ones_row32[:], rrow[:], start=True, stop=True)
            recipb = p_aux.tile([P, SB], FP32, name=f"recipb{sb}", tag="recipb")
            nc.vector.tensor_copy(recipb[:], ps_rb[:])
            return recipb

        def mm4_copy_out(sb, ps_list, ots, et0):
            for i, ps_o in enumerate(ps_list):
                o_t = p_ot.tile([P, SB], OT_DT, name=f"ot{sb}_{et0 + i}", tag="ot")
                nc.vector.tensor_copy(o_t[:], ps_o[:])
                ots.append(o_t)

        def emit_mm5(sb, ots, recipb):
            s0 = sb * SB
            for ft in range(FT):
                ps_f = p_psum.tile([P, SB], FP32, name=f"psf{sb}_{ft}", tag="ps")
                for ec in range(DC):
                    nc.tensor.matmul(
                        ps_f[:], fcw_sl(ec, ft), ots[ec][:],
                        start=(ec == 0), stop=(ec == DC - 1),
                    )
                tmp = p_tmp.tile([P, SB], FP32, name=f"tmp{sb}_{ft}", tag="tmp")
                nc.vector.tensor_mul(tmp[:], ps_f[:], recipb[:])
                o_out = p_oo.tile([P, SB], FP32, name=f"oo{sb}_{ft}", tag="oo")
                nc.scalar.activation(
                    o_out[:], tmp[:], AF.Relu,
                    bias=fcb_cols[:, ft : ft + 1], scale=1.0,
                )
                nc.sync.dma_start(outt_d[ft * P : (ft + 1) * P, s0 : s0 + SB], o_out[:])

        # ---------- steady-state blocks ----------
        maxbs[0] = emit_max_reduce(0)
        for sb in range(NSB):
            if sb + 2 < NSB:
                x2t_tiles[sb + 2] = issue_x2t(sb + 2)
            if sb + 1 < NSB:
                issue_x1tg((sb + 1) * NTG, 0)

            maxb = maxbs[sb]

            if sb + 1 < NSB:
                # PE chews MM2(sb+1) + denom matmuls while Scalar/DVE run exp(sb)
                ps_sum = p_psum1.tile([1, SB], FP32, name=f"psum{sb}", tag="ps1")
                for ti in range(TC):
                    emit_exp_tile(sb, ti, maxb)
                    tg, sub = divmod(ti, SB // P)
                    if sub == 0 and tg + 1 < NTG:
                        issue_x1tg((sb + 1) * NTG + tg + 1, tg + 1)
                    emit_mm2_tile(sb + 1, ti)
                    emit_sum_mm(sb, ti, ps_sum)
                rrow = emit_sum_recip(sb, ps_sum)
                if sb + 2 < NSB:
                    emit_mm1(sb + 2)
                # maxT(sb+1) here: the PE<->DVE ping-pong hides under MM4/MM5
                maxbs[sb + 1] = emit_max_reduce(sb + 1)
                ots = []
                for et in range(DC):
                    ps_o = p_psum.tile([P, SB], FP32, name=f"pso{sb}_{et}", tag="ps")
                    for ti in range(TC):
                        nc.tensor.matmul(
                            ps_o[:], x1_sl(ti, et), pb_tiles[sb][ti][:],
                            start=(ti == 0), stop=(ti == TC - 1),
                        )
                    mm4_copy_out(sb, [ps_o], ots, et)
                recipb = emit_recip_bcast(sb, rrow)
                emit_mm5(sb, ots, recipb)
            else:
                # last block: ti-outer MM4 on 7 held banks + denom bank; et7 after
                mm4_ps = [
                    p_psum1.tile([P, SB], FP32, name=f"pso{sb}_{et}", tag="ps1")
                    for et in range(4)
                ] + [
                    p_psum.tile([P, SB], FP32, name=f"pso{sb}_{et}", tag="ps")
                    for et in range(4, DC - 1)
                ]
                ps_sum = p_psum.tile([1, SB], FP32, name=f"psum{sb}", tag="ps")
                for ti in range(TC):
                    emit_exp_tile(sb, ti, maxb)
                    for et in range(DC - 1):
                        nc.tensor.matmul(
                            mm4_ps[et][:], x1_sl(ti, et), pb_tiles[sb][ti][:],
                            start=(ti == 0), stop=(ti == TC - 1),
                        )
                    emit_sum_mm(sb, ti, ps_sum)
                ots = []
                mm4_copy_out(sb, mm4_ps, ots, 0)
                rrow = emit_sum_recip(sb, ps_sum)
                ps_o7 = p_psum.tile([P, SB], FP32, name=f"pso{sb}_7", tag="ps")
                for ti in range(TC):
                    nc.tensor.matmul(
                        ps_o7[:], x1_sl(ti, DC - 1), pb_tiles[sb][ti][:],
                        start=(ti == 0), stop=(ti == TC - 1),
                    )
                mm4_copy_out(sb, [ps_o7], ots, DC - 1)
                recipb = emit_recip_bcast(sb, rrow)
                emit_mm5(sb, ots, recipb)

    nc.compile()
    return nc


_NC_CACHE = None


def _get_nc():
    global _NC_CACHE
    if _NC_CACHE is None:
        _NC_CACHE = build_nc()
    return _NC_CACHE


def make_in_maps(x1, x2, U, bias, fc_w, fc_b):
    import ml_dtypes

    x1 = np.ascontiguousarray(np.asarray(x1, dtype=np.float32))
    x2 = np.ascontiguousarray(np.asarray(x2, dtype=np.float32))
    U = np.ascontiguousarray(np.asarray(U, dtype=np.float32))
    bias = np.asarray(bias, dtype=np.float32)
    fc_w = np.asarray(fc_w, dtype=np.float32)
    fc_b = np.asarray(fc_b, dtype=np.float32)
    # relaid residents (same for every core)
    ug = np.ascontiguousarray(
        U.reshape(DC, P, D).transpose(1, 0, 2).reshape(P, DC * D)
    ).astype(np.float16)
    fcwg = np.ascontiguousarray(
        fc_w.T.reshape(DC, P, F).transpose(1, 0, 2).reshape(P, DC * F)
    ).astype(ml_dtypes.bfloat16)
    biasg = np.ascontiguousarray(bias.reshape(DC, P).T)
    fcbg = np.ascontiguousarray(fc_b.reshape(FT, P).T)
    in_maps = []
    for b in range(B):
        x1t = x1[b].T  # [D, S]
        x1tg = np.ascontiguousarray(
            x1t.reshape(DC, P, NTG, SB).transpose(1, 2, 0, 3).reshape(P, NTG * DC * SB)
        ).astype(np.float16)
        x1g = np.ascontiguousarray(
            x1[b].reshape(TC, P, D).transpose(1, 0, 2).reshape(P, TC * D)
        ).astype(ml_dtypes.bfloat16)
        in_maps.append(
            {
                "x1g": x1g,
                "x1tg": x1tg,
                "x2t": np.ascontiguousarray(x2[b].T).astype(np.float16),
                "ug": ug,
                "fcwg": fcwg,
                "biasg": biasg,
                "fcbg": fcbg,
            }
        )
    return in_maps


def kernel(x1, x2, U, bias, fc_w, fc_b):
    from concourse.bass_utils import run_bass_kernel_spmd

    nc = _get_nc()
    in_maps = make_in_maps(x1, x2, U, bias, fc_w, fc_b)
    res = run_bass_kernel_spmd(nc, in_maps, core_ids=list(range(B)))
    out = np.stack([np.ascontiguousarray(r["outt"].T) for r in res.results])
    return out.astype(np.float32)
